# revision 10
# baseline (speedup 1.0000x reference)
"""BitLinear inference kernel for Trainium2, SPMD over 8 NeuronCores.

Reference computation (per batch b of x[b] @ [T, D], kernel [D, F]):
  x_norm  = x * rsqrt(mean(x^2, -1) + 1e-5)
  x_scale = 127 / clip(max|x_norm|, 1e-5)          (per row)
  x_quant = round(x_norm * x_scale).clip(-128,127) / x_scale
  w_scale = mean|kernel|.clip(1e-5)
  w_quant = sign(kernel - mean(kernel)) * w_scale
  out     = (x_quant @ w_quant) / w_scale / x_scale

Algebra used here: w_scale cancels exactly, and with
  q    = round(x_norm * x_scale)  (integers in [-127, 127])
  s    = sign(kernel - mean(kernel))  (+-1)
  out  = (q @ s) * (max|x_norm| / 127)^2   per row.
The rsqrt never needs to be materialized: q = round(x * 127/max|x|)
(the row scale cancels inside the quant), and the output scale only
needs r^2 = 1/(mean(x^2)+1e-5), computed with the accurate DVE
reciprocal.  q and s are exactly representable in bf16 and the PE
accumulates in fp32, so the matmul is exact integer arithmetic.

Sharding: data-parallel — one batch of x per core, kernel replicated.
"""

import re
from contextlib import ExitStack

import numpy as np

import concourse.bass as bass
import concourse.mybir as mybir
import concourse.tile as tile
from concourse.tile import ScopedClock, VectorClock


# ---------------------------------------------------------------------------
# The walrus build in this container only accepts a single sync-wait per
# Drain instruction; TileContext's tail drain carries one wait per live
# semaphore.  Split it into one drain per semaphore.
# ---------------------------------------------------------------------------
def _drain_and_barrier_split(self, tick_clock, wait_clock):
    m = re.search(r"VectorClock\(\[([^\]]*)\]\)", repr(tick_clock.global_clock))
    vals = [int(v) for v in m.group(1).split(",")]
    procs = [(i, v) for i, v in enumerate(vals) if v > 0]
    for i, v in procs or [(0, 0)]:
        sub = VectorClock()
        sub.require_at_least(i, v)
        drain_inst = self.nc.sync.drain()
        wait_clock.add_sem_waits(drain_inst.ins, ScopedClock({None: sub}))

    self.nc.all_engine_barrier()
    assert self.sems is not None
    popped = self.nc._tile_sem_poison_stack.pop()
    assert popped is self._sem_poison
    self.nc.clear_and_free_semaphores(list(self.sems.allocated().values()))
    self.nc.all_engine_barrier()


def install_drain_patch():
    tile.TileContext._drain_and_barrier = _drain_and_barrier_split


def split_multi_waits(nc: bass.Bass, max_waits: int = 1):
    """The walrus in this container accepts at most one sync-wait per
    instruction.  Hoist extra waits onto NoOps injected just before the
    instruction on the same engine (engines execute their stream in order,
    so waiting on A then B sequentially == waiting on both)."""
    n_split = 0
    for fn in nc.m.functions:
        for bb in fn.blocks:
            insts = bb.instructions
            if not any(
                ins.sync_info is not None and len(ins.sync_info.on_wait or []) > max_waits
                for ins in insts
            ):
                continue
            out = []
            for ins in insts:
                si = ins.sync_info
                if si is not None and len(si.on_wait or []) > max_waits:
                    waits = list(si.on_wait)
                    for j, w in enumerate(waits[:-max_waits]):
                        nop = mybir.InstNoOp(name=f"{ins.name}-wsplit{j}", ins=[], outs=[])
                        nop.engine = ins.engine
                        nop.sync_info = mybir.SyncInfo(on_wait=[w], on_update=[])
                        nc.register_instruction(nop, overwrite=True)
                        out.append(nop)
                    ins.sync_info = mybir.SyncInfo(
                        on_wait=waits[-max_waits:], on_update=list(si.on_update or [])
                    )
                    n_split += 1
                out.append(ins)
            bb.instructions = out
    return n_split


MAGIC = float(1.5 * 2.0**23)  # keeps v+MAGIC in [2^23, 2^24) for |v| <= 2^22 -> RNE to integer
F32 = mybir.dt.float32
BF16 = mybir.dt.bfloat16
P = 128


def build_bitlinear(nc: bass.Bass, T=4096, D=4096, F=4096, TG=1024, FC=512):
    """Emit the per-core program: x [T, D] f32, w [D, F] f32 -> out [T, F] f32."""
    AF = mybir.ActivationFunctionType
    KB = D // P          # contraction slices
    NG = T // TG         # token groups
    GB = TG // P         # 128-token blocks per group
    NFC = F // FC        # output-feature chunks
    SUB = 512            # bn_stats subgroup width
    NSUB = D // SUB

    x_in = nc.dram_tensor("x", [T, D], F32, kind="ExternalInput")
    w_in = nc.dram_tensor("w", [D, F], F32, kind="ExternalInput")
    out = nc.dram_tensor("out", [T, F], F32, kind="ExternalOutput")

    with tile.TileContext(nc) as tc, ExitStack() as ctx:
        xp = ctx.enter_context(tc.tile_pool(name="xp", bufs=2))
        qbp = ctx.enter_context(tc.tile_pool(name="qbp", bufs=3))
        qtp = ctx.enter_context(tc.tile_pool(name="qtp", bufs=1))
        sp = ctx.enter_context(tc.tile_pool(name="sp", bufs=2))
        w2p = ctx.enter_context(tc.tile_pool(name="w2p", bufs=4))
        stg = ctx.enter_context(tc.tile_pool(name="stg", bufs=3))
        st = ctx.enter_context(tc.tile_pool(name="st", bufs=6))
        postp = ctx.enter_context(tc.tile_pool(name="postp", bufs=2 * GB))
        singles = ctx.enter_context(tc.tile_pool(name="singles", bufs=1))
        psmm = ctx.enter_context(tc.tile_pool(name="psmm", bufs=4, space="PSUM"))
        psw = ctx.enter_context(tc.tile_pool(name="psw", bufs=1, space="PSUM"))
        dram = ctx.enter_context(tc.tile_pool(name="dram", bufs=1, space="DRAM"))

        s_dram = dram.tile([NFC, P, KB, FC], BF16)

        # ---- W pass 1: total sum of w (for the global mean) ----
        colsum = singles.tile([P, KB * NFC], F32)
        for rb in range(KB):
            for fc in range(NFC):
                wt = w2p.tile([P, FC], F32, tag="wtile")
                nc.sync.dma_start(out=wt, in_=w_in[rb * P:(rb + 1) * P, fc * FC:(fc + 1) * FC])
                nc.vector.reduce_sum(
                    out=colsum[:, rb * NFC + fc : rb * NFC + fc + 1],
                    in_=wt,
                    axis=mybir.AxisListType.X,
                )
        rowsum = st.tile([P, 1], F32)
        nc.vector.reduce_sum(out=rowsum, in_=colsum, axis=mybir.AxisListType.X)
        ones_col = singles.tile([P, 1], F32)
        nc.vector.memset(ones_col, 1.0)
        ones_row = singles.tile([1, P], F32)
        nc.vector.memset(ones_row, 1.0)
        magic_bias = singles.tile([P, 1], F32)
        nc.vector.memset(magic_bias, MAGIC)
        ps_scalar = psw.tile([1, 1], F32)
        nc.tensor.matmul(ps_scalar, lhsT=rowsum, rhs=ones_col, start=True, stop=True)
        sb_scalar = st.tile([1, 1], F32)
        nc.vector.tensor_copy(sb_scalar, ps_scalar)
        ps_bc = psw.tile([P, 1], F32)
        nc.tensor.matmul(ps_bc, lhsT=ones_row, rhs=sb_scalar, start=True, stop=True)
        neg_wmean = singles.tile([P, 1], F32)
        nc.scalar.activation(neg_wmean, ps_bc, AF.Copy, bias=0.0, scale=-1.0 / (D * F))

        # ---- W pass 2: s = sign(w - mean) as bf16, chunk-major in DRAM ----
        for fc in range(NFC):
            s_sb = sp.tile([P, KB, FC], BF16, tag="schunk")
            for rb in range(KB):
                wt = w2p.tile([P, FC], F32, tag="wtile")
                nc.sync.dma_start(out=wt, in_=w_in[rb * P:(rb + 1) * P, fc * FC:(fc + 1) * FC])
                nc.scalar.activation(out=s_sb[:, rb, :], in_=wt, func=AF.Sign, bias=neg_wmean, scale=1.0)
            nc.sync.dma_start(out=s_dram[fc], in_=s_sb)

        # ---- main loop over token groups ----
        for g in range(NG):
            qT = qtp.tile([P, KB, GB, P], BF16)
            posts = []
            for tb in range(GB):
                b = g * GB + tb
                xt = xp.tile([P, D], F32)
                nc.sync.dma_start(out=xt, in_=x_in[b * P:(b + 1) * P, :])

                # mean(x^2) via bn_stats (no main output needed)
                stats6 = st.tile([P, NSUB, 6], F32)
                for i in range(NSUB):
                    nc.vector.bn_stats(out=stats6[:, i, :], in_=xt[:, i * SUB:(i + 1) * SUB])
                mv = st.tile([P, 2], F32)
                nc.vector.bn_aggr(out=mv, in_=stats6)
                msq = st.tile([P, 1], F32)
                nc.vector.tensor_mul(msq, mv[:, 0:1], mv[:, 0:1])
                v0 = st.tile([P, 1], F32)
                nc.vector.tensor_add(v0, msq, mv[:, 1:2])
                v1 = st.tile([P, 1], F32)
                nc.vector.tensor_scalar_add(v1, v0, 1e-5)
                r2 = st.tile([P, 1], F32)
                nc.vector.reciprocal(r2, v1)

                am = st.tile([P, 1], F32)
                nc.vector.tensor_reduce(
                    out=am, in_=xt, axis=mybir.AxisListType.X,
                    op=mybir.AluOpType.max, apply_absolute_value=True,
                )
                am2 = st.tile([P, 1], F32)
                nc.vector.tensor_mul(am2, am, am)
                a2 = st.tile([P, 1], F32)
                nc.vector.tensor_mul(a2, am2, r2)
                post = postp.tile([P, 1], F32)
                nc.vector.tensor_scalar(
                    out=post, in0=a2, scalar1=1e-10, scalar2=1.0 / (127.0 * 127.0),
                    op0=mybir.AluOpType.max, op1=mybir.AluOpType.mult,
                )
                w1 = st.tile([P, 1], F32)
                nc.vector.tensor_scalar(
                    out=w1, in0=am, scalar1=1e-30, scalar2=1.0 / 127.0,
                    op0=mybir.AluOpType.max, op1=mybir.AluOpType.mult,
                )
                cc = st.tile([P, 1], F32)
                nc.vector.reciprocal(cc, w1)

                # q = round(x * c) via the 2^23 magic-number trick (RNE)
                nc.scalar.activation(out=xt, in_=xt, func=AF.Identity, bias=magic_bias, scale=cc)
                qb = qbp.tile([P, D], BF16)
                nc.vector.tensor_scalar_add(qb, xt, -MAGIC)

                for k in range(KB):
                    nc.scalar.dma_start_transpose(out=qT[:, k, tb, :], in_=qb[:, k * P:(k + 1) * P])
                posts.append(post)

            for fc in range(NFC):
                s_sb = sp.tile([P, KB, FC], BF16, tag="schunk")
                nc.sync.dma_start(out=s_sb, in_=s_dram[fc])
                for tb in range(GB):
                    ps = psmm.tile([P, FC], F32)
                    for k in range(KB):
                        nc.tensor.matmul(
                            ps,
                            lhsT=qT[:, k, tb, :],
                            rhs=s_sb[:, k, :],
                            start=(k == 0),
                            stop=(k == KB - 1),
                        )
                    so = stg.tile([P, FC], F32)
                    nc.scalar.activation(out=so, in_=ps, func=AF.Copy, bias=0.0, scale=posts[tb])
                    b = g * GB + tb
                    nc.sync.dma_start(
                        out=out[b * P:(b + 1) * P, fc * FC:(fc + 1) * FC], in_=so
                    )
    return nc


_N_CORES = 8
_BATCH = 8
_T = 4096
_D = 4096
_F = 4096


def _ensure_axon_hooks_module():
    """bass_utils imports antenv.axon_hooks when BASS_TRACE is set; the
    module is absent in this image.  Provide a stub so tracing degrades
    gracefully instead of crashing (a real hook may already be installed)."""
    import sys
    import types

    try:
        import antenv.axon_hooks  # noqa: F401
    except ImportError:
        mod = types.ModuleType("antenv.axon_hooks")
        mod._hook = None
        mod.set_axon_ntff_profile_hook = lambda h: setattr(mod, "_hook", h)
        mod.get_axon_ntff_profile_hook = lambda: mod._hook
        sys.modules["antenv.axon_hooks"] = mod


def kernel(x: np.ndarray, kernel: np.ndarray) -> np.ndarray:
    from concourse.bass_utils import run_bass_kernel_spmd

    _ensure_axon_hooks_module()
    install_drain_patch()
    nc = bass.Bass()
    build_bitlinear(nc, T=_T, D=_D, F=_F, TG=1024, FC=512)
    split_multi_waits(nc)

    x = np.ascontiguousarray(np.asarray(x, dtype=np.float32))
    w = np.ascontiguousarray(np.asarray(kernel, dtype=np.float32))
    assert x.shape == (_BATCH, _T, _D) and w.shape == (_D, _F)

    in_maps = [{"x": x[b], "w": w} for b in range(_N_CORES)]
    res = run_bass_kernel_spmd(nc, in_maps, list(range(_N_CORES)))
    global _last_results
    _last_results = res
    return np.stack([res.results[i]["out"] for i in range(_N_CORES)], axis=0)


_last_results = None


# revision 15
# speedup vs baseline: 1.4772x; 1.4772x over previous
"""BitLinear inference kernel for Trainium2, SPMD over 8 NeuronCores.

Reference computation (per batch b of x[b] @ [T, D], kernel [D, F]):
  x_norm  = x * rsqrt(mean(x^2, -1) + 1e-5)
  x_scale = 127 / clip(max|x_norm|, 1e-5)          (per row)
  x_quant = round(x_norm * x_scale).clip(-128,127) / x_scale
  w_scale = mean|kernel|.clip(1e-5)
  w_quant = sign(kernel - mean(kernel)) * w_scale
  out     = (x_quant @ w_quant) / w_scale / x_scale

Algebra used here: w_scale cancels exactly, and with
  q    = round(x_norm * x_scale)  (integers in [-127, 127])
  s    = sign(kernel - mean(kernel))  (+-1)
  out  = (q @ s) * (max|x_norm| / 127)^2   per row.
The rsqrt never needs to be materialized: q = round(x * 127/max|x|)
(the row scale cancels inside the quant), and the output scale only
needs r^2 = 1/(mean(x^2)+1e-5), computed with the accurate DVE
reciprocal.  q and s are exactly representable in bf16 and the PE
accumulates in fp32, so the matmul is exact integer arithmetic.

Sharding: data-parallel — one batch of x per core, kernel replicated.
"""

import re
from contextlib import ExitStack

import numpy as np

import concourse.bass as bass
import concourse.mybir as mybir
import concourse.tile as tile
from concourse.tile import ScopedClock, VectorClock


# ---------------------------------------------------------------------------
# The walrus build in this container only accepts a single sync-wait per
# Drain instruction; TileContext's tail drain carries one wait per live
# semaphore.  Split it into one drain per semaphore.
# ---------------------------------------------------------------------------
def _drain_and_barrier_split(self, tick_clock, wait_clock):
    m = re.search(r"VectorClock\(\[([^\]]*)\]\)", repr(tick_clock.global_clock))
    vals = [int(v) for v in m.group(1).split(",")]
    procs = [(i, v) for i, v in enumerate(vals) if v > 0]
    for i, v in procs or [(0, 0)]:
        sub = VectorClock()
        sub.require_at_least(i, v)
        drain_inst = self.nc.sync.drain()
        wait_clock.add_sem_waits(drain_inst.ins, ScopedClock({None: sub}))

    self.nc.all_engine_barrier()
    assert self.sems is not None
    popped = self.nc._tile_sem_poison_stack.pop()
    assert popped is self._sem_poison
    self.nc.clear_and_free_semaphores(list(self.sems.allocated().values()))
    self.nc.all_engine_barrier()


def install_drain_patch():
    tile.TileContext._drain_and_barrier = _drain_and_barrier_split


def split_multi_waits(nc: bass.Bass, max_waits: int = 1):
    """The walrus in this container accepts at most one sync-wait per
    instruction.  Hoist extra waits onto NoOps injected just before the
    instruction on the same engine (engines execute their stream in order,
    so waiting on A then B sequentially == waiting on both)."""
    n_split = 0
    for fn in nc.m.functions:
        for bb in fn.blocks:
            insts = bb.instructions
            if not any(
                ins.sync_info is not None and len(ins.sync_info.on_wait or []) > max_waits
                for ins in insts
            ):
                continue
            out = []
            for ins in insts:
                si = ins.sync_info
                if si is not None and len(si.on_wait or []) > max_waits:
                    waits = list(si.on_wait)
                    for j, w in enumerate(waits[:-max_waits]):
                        nop = mybir.InstNoOp(name=f"{ins.name}-wsplit{j}", ins=[], outs=[])
                        nop.engine = ins.engine
                        nop.sync_info = mybir.SyncInfo(on_wait=[w], on_update=[])
                        nc.register_instruction(nop, overwrite=True)
                        out.append(nop)
                    ins.sync_info = mybir.SyncInfo(
                        on_wait=waits[-max_waits:], on_update=list(si.on_update or [])
                    )
                    n_split += 1
                out.append(ins)
            bb.instructions = out
    return n_split


MAGIC = float(1.5 * 2.0**23)  # keeps v+MAGIC in [2^23, 2^24) for |v| <= 2^22 -> RNE to integer
F32 = mybir.dt.float32
BF16 = mybir.dt.bfloat16
P = 128


def build_bitlinear(nc: bass.Bass, T=4096, D=4096, F=4096, TG=1024, FC=512):
    """Emit the per-core program: x [T, D] f32, w [D, F] f32 -> out [T, F] f32."""
    AF = mybir.ActivationFunctionType
    KB = D // P          # contraction slices
    NG = T // TG         # token groups
    GB = TG // P         # 128-token blocks per group
    NFC = F // FC        # output-feature chunks
    SUB = 512            # bn_stats subgroup width
    NSUB = D // SUB

    x_in = nc.dram_tensor("x", [T, D], F32, kind="ExternalInput")
    w_in = nc.dram_tensor("w", [D, F], F32, kind="ExternalInput")
    out = nc.dram_tensor("out", [T, F], F32, kind="ExternalOutput")

    with tile.TileContext(nc) as tc, ExitStack() as ctx:
        xp = ctx.enter_context(tc.tile_pool(name="xp", bufs=2))
        qbp = ctx.enter_context(tc.tile_pool(name="qbp", bufs=3))
        qtp = ctx.enter_context(tc.tile_pool(name="qtp", bufs=1))
        sp = ctx.enter_context(tc.tile_pool(name="sp", bufs=2))
        w2p = ctx.enter_context(tc.tile_pool(name="w2p", bufs=4))
        stg = ctx.enter_context(tc.tile_pool(name="stg", bufs=3))
        st = ctx.enter_context(tc.tile_pool(name="st", bufs=6))
        postp = ctx.enter_context(tc.tile_pool(name="postp", bufs=2 * GB))
        singles = ctx.enter_context(tc.tile_pool(name="singles", bufs=1))
        psmm = ctx.enter_context(tc.tile_pool(name="psmm", bufs=4, space="PSUM"))
        psw = ctx.enter_context(tc.tile_pool(name="psw", bufs=1, space="PSUM"))
        dram = ctx.enter_context(tc.tile_pool(name="dram", bufs=1, space="DRAM"))

        s_drams = [
            dram.tile([P, KB, FC], BF16, tag=f"sd{fc}", name=f"sd{fc}")
            for fc in range(NFC)
        ]

        # ---- W pass 1: total sum of w (for the global mean) ----
        # Contiguous full-width row-block reads (2 MiB DMAs).
        colsum = singles.tile([P, KB], F32)
        for rb in range(KB):
            wt = xp.tile([P, F], F32, tag="xt")
            nc.sync.dma_start(out=wt, in_=w_in[rb * P:(rb + 1) * P, :])
            nc.vector.reduce_sum(
                out=colsum[:, rb : rb + 1], in_=wt, axis=mybir.AxisListType.X
            )
        rowsum = st.tile([P, 1], F32)
        nc.vector.reduce_sum(out=rowsum, in_=colsum, axis=mybir.AxisListType.X)
        ones_col = singles.tile([P, 1], F32)
        nc.vector.memset(ones_col, 1.0)
        ones_row = singles.tile([1, P], F32)
        nc.vector.memset(ones_row, 1.0)
        magic_bias = singles.tile([P, 1], F32)
        nc.vector.memset(magic_bias, MAGIC)
        ps_scalar = psw.tile([1, 1], F32)
        nc.tensor.matmul(ps_scalar, lhsT=rowsum, rhs=ones_col, start=True, stop=True)
        sb_scalar = st.tile([1, 1], F32)
        nc.vector.tensor_copy(sb_scalar, ps_scalar)
        ps_bc = psw.tile([P, 1], F32)
        nc.tensor.matmul(ps_bc, lhsT=ones_row, rhs=sb_scalar, start=True, stop=True)
        neg_wmean = singles.tile([P, 1], F32)
        nc.scalar.activation(neg_wmean, ps_bc, AF.Copy, bias=0.0, scale=-1.0 / (D * F))

        # ---- W pass 2: s = sign(w - mean) as bf16, chunk-major in DRAM ----
        for fc in range(NFC):
            s_sb = sp.tile([P, KB, FC], BF16, tag="schunk")
            for rb in range(KB):
                wt = w2p.tile([P, FC], F32, tag="wtile")
                nc.sync.dma_start(out=wt, in_=w_in[rb * P:(rb + 1) * P, fc * FC:(fc + 1) * FC])
                nc.scalar.activation(out=s_sb[:, rb, :], in_=wt, func=AF.Sign, bias=neg_wmean, scale=1.0)
            nc.sync.dma_start(out=s_drams[fc][:, :, :], in_=s_sb)

        # ---- main loop over token groups ----
        for g in range(NG):
            qT = qtp.tile([P, KB, GB, P], BF16)
            posts = []
            for tb in range(GB):
                b = g * GB + tb
                xt = xp.tile([P, D], F32)
                nc.sync.dma_start(out=xt, in_=x_in[b * P:(b + 1) * P, :])

                # mean(x^2) via bn_stats (no main output needed)
                stats6 = st.tile([P, NSUB, 6], F32)
                for i in range(NSUB):
                    nc.vector.bn_stats(out=stats6[:, i, :], in_=xt[:, i * SUB:(i + 1) * SUB])
                mv = st.tile([P, 2], F32)
                nc.vector.bn_aggr(out=mv, in_=stats6)
                msq = st.tile([P, 1], F32)
                nc.vector.tensor_mul(msq, mv[:, 0:1], mv[:, 0:1])
                v0 = st.tile([P, 1], F32)
                nc.vector.tensor_add(v0, msq, mv[:, 1:2])
                v1 = st.tile([P, 1], F32)
                nc.vector.tensor_scalar_add(v1, v0, 1e-5)
                r2 = st.tile([P, 1], F32)
                nc.vector.reciprocal(r2, v1)

                am = st.tile([P, 1], F32)
                nc.vector.tensor_reduce(
                    out=am, in_=xt, axis=mybir.AxisListType.X,
                    op=mybir.AluOpType.max, apply_absolute_value=True,
                )
                am2 = st.tile([P, 1], F32)
                nc.vector.tensor_mul(am2, am, am)
                a2 = st.tile([P, 1], F32)
                nc.vector.tensor_mul(a2, am2, r2)
                post = postp.tile([P, 1], F32)
                nc.vector.tensor_scalar(
                    out=post, in0=a2, scalar1=1e-10, scalar2=1.0 / (127.0 * 127.0),
                    op0=mybir.AluOpType.max, op1=mybir.AluOpType.mult,
                )
                w1 = st.tile([P, 1], F32)
                nc.vector.tensor_scalar(
                    out=w1, in0=am, scalar1=1e-30, scalar2=1.0 / 127.0,
                    op0=mybir.AluOpType.max, op1=mybir.AluOpType.mult,
                )
                cc = st.tile([P, 1], F32)
                nc.vector.reciprocal(cc, w1)

                # q = round(x * c) via the 2^23 magic-number trick (RNE)
                nc.scalar.activation(out=xt, in_=xt, func=AF.Identity, bias=magic_bias, scale=cc)
                qb = qbp.tile([P, D], BF16)
                nc.vector.tensor_scalar_add(qb, xt, -MAGIC)

                # one batched xbar transpose per block: qT[p, k, tb, t] = qb[t, k*P+p]
                nc.scalar.dma_start_transpose(out=qT[:, :, tb, :], in_=qb[:, :])
                posts.append(post)

            for fc in range(NFC):
                s_sb = sp.tile([P, KB, FC], BF16, tag="schunk")
                nc.sync.dma_start(out=s_sb, in_=s_drams[fc][:, :, :])
                for tb in range(GB):
                    ps = psmm.tile([P, FC], F32)
                    for k in range(KB):
                        nc.tensor.matmul(
                            ps,
                            lhsT=qT[:, k, tb, :],
                            rhs=s_sb[:, k, :],
                            start=(k == 0),
                            stop=(k == KB - 1),
                        )
                    so = stg.tile([P, FC], F32)
                    nc.scalar.activation(out=so, in_=ps, func=AF.Copy, bias=0.0, scale=posts[tb])
                    b = g * GB + tb
                    nc.sync.dma_start(
                        out=out[b * P:(b + 1) * P, fc * FC:(fc + 1) * FC], in_=so
                    )
    return nc


_N_CORES = 8
_BATCH = 8
_T = 4096
_D = 4096
_F = 4096


def _ensure_axon_hooks_module():
    """bass_utils imports antenv.axon_hooks when BASS_TRACE is set; the
    module is absent in this image.  Provide a stub so tracing degrades
    gracefully instead of crashing (a real hook may already be installed)."""
    import sys
    import types

    try:
        import antenv.axon_hooks  # noqa: F401
    except ImportError:
        mod = types.ModuleType("antenv.axon_hooks")
        mod._hook = None
        mod.set_axon_ntff_profile_hook = lambda h: setattr(mod, "_hook", h)
        mod.get_axon_ntff_profile_hook = lambda: mod._hook
        sys.modules["antenv.axon_hooks"] = mod


def kernel(x: np.ndarray, kernel: np.ndarray) -> np.ndarray:
    from concourse.bass_utils import run_bass_kernel_spmd

    _ensure_axon_hooks_module()
    install_drain_patch()
    nc = bass.Bass()
    build_bitlinear(nc, T=_T, D=_D, F=_F, TG=1024, FC=512)
    split_multi_waits(nc)

    x = np.ascontiguousarray(np.asarray(x, dtype=np.float32))
    w = np.ascontiguousarray(np.asarray(kernel, dtype=np.float32))
    assert x.shape == (_BATCH, _T, _D) and w.shape == (_D, _F)

    in_maps = [{"x": x[b], "w": w} for b in range(_N_CORES)]
    res = run_bass_kernel_spmd(nc, in_maps, list(range(_N_CORES)))
    global _last_results
    _last_results = res
    return np.stack([res.results[i]["out"] for i in range(_N_CORES)], axis=0)


_last_results = None


# revision 20
# speedup vs baseline: 1.5381x; 1.0412x over previous
"""BitLinear inference kernel for Trainium2, SPMD over 8 NeuronCores.

Reference computation (per batch b of x[b] @ [T, D], kernel [D, F]):
  x_norm  = x * rsqrt(mean(x^2, -1) + 1e-5)
  x_scale = 127 / clip(max|x_norm|, 1e-5)          (per row)
  x_quant = round(x_norm * x_scale).clip(-128,127) / x_scale
  w_scale = mean|kernel|.clip(1e-5)
  w_quant = sign(kernel - mean(kernel)) * w_scale
  out     = (x_quant @ w_quant) / w_scale / x_scale

Algebra used here: w_scale cancels exactly, and with
  q    = round(x_norm * x_scale)  (integers in [-127, 127])
  s    = sign(kernel - mean(kernel))  (+-1)
  out  = (q @ s) * (max|x_norm| / 127)^2   per row.
The rsqrt never needs to be materialized: q = round(x * 127/max|x|)
(the row scale cancels inside the quant), and the output scale only
needs r^2 = 1/(mean(x^2)+1e-5), computed with the accurate DVE
reciprocal.  q and s are exactly representable in bf16 and the PE
accumulates in fp32, so the matmul is exact integer arithmetic.

Sharding: data-parallel — one batch of x per core, kernel replicated.
"""

import re
from contextlib import ExitStack

import numpy as np

import concourse.bass as bass
import concourse.mybir as mybir
import concourse.tile as tile
from concourse.tile import ScopedClock, VectorClock


# ---------------------------------------------------------------------------
# The walrus build in this container only accepts a single sync-wait per
# Drain instruction; TileContext's tail drain carries one wait per live
# semaphore.  Split it into one drain per semaphore.
# ---------------------------------------------------------------------------
def _drain_and_barrier_split(self, tick_clock, wait_clock):
    m = re.search(r"VectorClock\(\[([^\]]*)\]\)", repr(tick_clock.global_clock))
    vals = [int(v) for v in m.group(1).split(",")]
    procs = [(i, v) for i, v in enumerate(vals) if v > 0]
    for i, v in procs or [(0, 0)]:
        sub = VectorClock()
        sub.require_at_least(i, v)
        drain_inst = self.nc.sync.drain()
        wait_clock.add_sem_waits(drain_inst.ins, ScopedClock({None: sub}))

    self.nc.all_engine_barrier()
    assert self.sems is not None
    popped = self.nc._tile_sem_poison_stack.pop()
    assert popped is self._sem_poison
    self.nc.clear_and_free_semaphores(list(self.sems.allocated().values()))
    self.nc.all_engine_barrier()


def install_drain_patch():
    tile.TileContext._drain_and_barrier = _drain_and_barrier_split


def split_multi_waits(nc: bass.Bass, max_waits: int = 1):
    """The walrus in this container accepts at most one sync-wait per
    instruction.  Hoist extra waits onto NoOps injected just before the
    instruction on the same engine (engines execute their stream in order,
    so waiting on A then B sequentially == waiting on both)."""
    n_split = 0
    for fn in nc.m.functions:
        for bb in fn.blocks:
            insts = bb.instructions
            if not any(
                ins.sync_info is not None and len(ins.sync_info.on_wait or []) > max_waits
                for ins in insts
            ):
                continue
            out = []
            for ins in insts:
                si = ins.sync_info
                if si is not None and len(si.on_wait or []) > max_waits:
                    waits = list(si.on_wait)
                    for j, w in enumerate(waits[:-max_waits]):
                        nop = mybir.InstNoOp(name=f"{ins.name}-wsplit{j}", ins=[], outs=[])
                        nop.engine = ins.engine
                        nop.sync_info = mybir.SyncInfo(on_wait=[w], on_update=[])
                        nc.register_instruction(nop, overwrite=True)
                        out.append(nop)
                    ins.sync_info = mybir.SyncInfo(
                        on_wait=waits[-max_waits:], on_update=list(si.on_update or [])
                    )
                    n_split += 1
                out.append(ins)
            bb.instructions = out
    return n_split


MAGIC = float(1.5 * 2.0**23)  # keeps v+MAGIC in [2^23, 2^24) for |v| <= 2^22 -> RNE to integer
F32 = mybir.dt.float32
BF16 = mybir.dt.bfloat16
P = 128


def build_bitlinear(nc: bass.Bass, T=4096, D=4096, F=4096, TG=1024, FC=512, world=8):
    """Emit the per-core program: x [T, D] f32, w [D, F] f32 -> out [T, F] f32.

    wslice [D/world, F] is this core's row-slice of w; partial sums are
    AllReduced so each core only reads 1/world of w for the global mean."""
    AF = mybir.ActivationFunctionType
    KB = D // P          # contraction slices
    NG = T // TG         # token groups
    GB = TG // P         # 128-token blocks per group
    NFC = F // FC        # output-feature chunks
    SUB = 512            # bn_stats subgroup width
    NSUB = D // SUB
    WRB = D // world // P  # row-blocks in wslice

    x_in = nc.dram_tensor("x", [T, D], F32, kind="ExternalInput")
    w_in = nc.dram_tensor("w", [D, F], F32, kind="ExternalInput")
    ws_in = nc.dram_tensor("wslice", [D // world, F], F32, kind="ExternalInput")
    out = nc.dram_tensor("out", [T, F], F32, kind="ExternalOutput")

    with tile.TileContext(nc) as tc, ExitStack() as ctx:
        xp = ctx.enter_context(tc.tile_pool(name="xp", bufs=2))
        qbp = ctx.enter_context(tc.tile_pool(name="qbp", bufs=3))
        qtp = ctx.enter_context(tc.tile_pool(name="qtp", bufs=1))
        sp = ctx.enter_context(tc.tile_pool(name="sp", bufs=2))
        w2p = ctx.enter_context(tc.tile_pool(name="w2p", bufs=4))
        stg = ctx.enter_context(tc.tile_pool(name="stg", bufs=3))
        st = ctx.enter_context(tc.tile_pool(name="st", bufs=6))
        postp = ctx.enter_context(tc.tile_pool(name="postp", bufs=2 * GB))
        singles = ctx.enter_context(tc.tile_pool(name="singles", bufs=1))
        psmm = ctx.enter_context(tc.tile_pool(name="psmm", bufs=4, space="PSUM"))
        psw = ctx.enter_context(tc.tile_pool(name="psw", bufs=1, space="PSUM"))
        dram = ctx.enter_context(tc.tile_pool(name="dram", bufs=1, space="DRAM"))

        s_drams = [
            dram.tile([P, KB, FC], BF16, tag=f"sd{fc}", name=f"sd{fc}")
            for fc in range(NFC)
        ]

        # ---- W pass 1: global mean via per-core partial sums + AllReduce ----
        # Contiguous full-width row-block reads (2 MiB DMAs) of this core's slice.
        colsum = singles.tile([P, WRB], F32)
        for rb in range(WRB):
            wt = xp.tile([P, F], F32, tag="xt")
            nc.sync.dma_start(out=wt, in_=ws_in[rb * P:(rb + 1) * P, :])
            nc.vector.reduce_sum(
                out=colsum[:, rb : rb + 1], in_=wt, axis=mybir.AxisListType.X
            )
        if world > 1:
            cc_in = dram.tile([P, WRB], F32, name="cc_in")
            cc_out = dram.tile([P, WRB], F32, name="cc_out", addr_space="Shared")
            nc.gpsimd.dma_start(out=cc_in[:, :], in_=colsum)
            nc.gpsimd.collective_compute(
                "AllReduce",
                mybir.AluOpType.add,
                replica_groups=[list(range(world))],
                ins=[cc_in[:, :]],
                outs=[cc_out[:, :]],
            )
            colsum_all = st.tile([P, WRB], F32)
            nc.gpsimd.dma_start(out=colsum_all, in_=cc_out[:, :])
        else:
            colsum_all = colsum
        rowsum = st.tile([P, 1], F32)
        nc.vector.reduce_sum(out=rowsum, in_=colsum_all, axis=mybir.AxisListType.X)
        ones_col = singles.tile([P, 1], F32)
        nc.vector.memset(ones_col, 1.0)
        ones_row = singles.tile([1, P], F32)
        nc.vector.memset(ones_row, 1.0)
        magic_bias = singles.tile([P, 1], F32)
        nc.vector.memset(magic_bias, MAGIC)
        ps_scalar = psw.tile([1, 1], F32)
        nc.tensor.matmul(ps_scalar, lhsT=rowsum, rhs=ones_col, start=True, stop=True)
        sb_scalar = st.tile([1, 1], F32)
        nc.vector.tensor_copy(sb_scalar, ps_scalar)
        ps_bc = psw.tile([P, 1], F32)
        nc.tensor.matmul(ps_bc, lhsT=ones_row, rhs=sb_scalar, start=True, stop=True)
        neg_wmean = singles.tile([P, 1], F32)
        nc.scalar.activation(neg_wmean, ps_bc, AF.Copy, bias=0.0, scale=-1.0 / (D * F))

        # ---- main loop over token groups ----
        # Group 0 computes s = sign(w - mean) chunk-by-chunk on the fly
        # (feeding its own matmuls directly) and writes each chunk back to
        # DRAM; groups 1+ stream the cached bf16 chunks.
        for g in range(NG):
            qT = qtp.tile([P, KB, GB, P], BF16)
            posts = []
            for tb in range(GB):
                b = g * GB + tb
                xt = xp.tile([P, D], F32)
                nc.sync.dma_start(out=xt, in_=x_in[b * P:(b + 1) * P, :])

                # mean(x^2) via bn_stats (no main output needed)
                stats6 = st.tile([P, NSUB, 6], F32)
                for i in range(NSUB):
                    nc.vector.bn_stats(out=stats6[:, i, :], in_=xt[:, i * SUB:(i + 1) * SUB])
                mv = st.tile([P, 2], F32)
                nc.vector.bn_aggr(out=mv, in_=stats6)
                msq = st.tile([P, 1], F32)
                nc.vector.tensor_mul(msq, mv[:, 0:1], mv[:, 0:1])
                v0 = st.tile([P, 1], F32)
                nc.vector.tensor_add(v0, msq, mv[:, 1:2])
                v1 = st.tile([P, 1], F32)
                nc.vector.tensor_scalar_add(v1, v0, 1e-5)
                r2 = st.tile([P, 1], F32)
                nc.vector.reciprocal(r2, v1)

                am = st.tile([P, 1], F32)
                nc.vector.tensor_reduce(
                    out=am, in_=xt, axis=mybir.AxisListType.X,
                    op=mybir.AluOpType.max, apply_absolute_value=True,
                )
                am2 = st.tile([P, 1], F32)
                nc.vector.tensor_mul(am2, am, am)
                a2 = st.tile([P, 1], F32)
                nc.vector.tensor_mul(a2, am2, r2)
                post = postp.tile([P, 1], F32)
                nc.vector.tensor_scalar(
                    out=post, in0=a2, scalar1=1e-10, scalar2=1.0 / (127.0 * 127.0),
                    op0=mybir.AluOpType.max, op1=mybir.AluOpType.mult,
                )
                w1 = st.tile([P, 1], F32)
                nc.vector.tensor_scalar(
                    out=w1, in0=am, scalar1=1e-30, scalar2=1.0 / 127.0,
                    op0=mybir.AluOpType.max, op1=mybir.AluOpType.mult,
                )
                cc = st.tile([P, 1], F32)
                nc.vector.reciprocal(cc, w1)

                # q = round(x * c) via the 2^23 magic-number trick (RNE)
                nc.scalar.activation(out=xt, in_=xt, func=AF.Identity, bias=magic_bias, scale=cc)
                qb = qbp.tile([P, D], BF16)
                nc.vector.tensor_scalar_add(qb, xt, -MAGIC)

                # one batched xbar transpose per block: qT[p, k, tb, t] = qb[t, k*P+p]
                nc.scalar.dma_start_transpose(out=qT[:, :, tb, :], in_=qb[:, :])
                posts.append(post)

            for fc in range(NFC):
                s_sb = sp.tile([P, KB, FC], BF16, tag="schunk")
                if g == 0:
                    for rb in range(KB):
                        wt2 = w2p.tile([P, FC], F32, tag="wtile")
                        nc.sync.dma_start(
                            out=wt2,
                            in_=w_in[rb * P:(rb + 1) * P, fc * FC:(fc + 1) * FC],
                        )
                        nc.scalar.activation(
                            out=s_sb[:, rb, :], in_=wt2, func=AF.Sign,
                            bias=neg_wmean, scale=1.0,
                        )
                    nc.sync.dma_start(out=s_drams[fc][:, :, :], in_=s_sb)
                else:
                    nc.sync.dma_start(out=s_sb, in_=s_drams[fc][:, :, :])
                for tb in range(GB):
                    ps = psmm.tile([P, FC], F32)
                    for k in range(KB):
                        nc.tensor.matmul(
                            ps,
                            lhsT=qT[:, k, tb, :],
                            rhs=s_sb[:, k, :],
                            start=(k == 0),
                            stop=(k == KB - 1),
                        )
                    so = stg.tile([P, FC], F32)
                    nc.scalar.activation(out=so, in_=ps, func=AF.Copy, bias=0.0, scale=posts[tb])
                    b = g * GB + tb
                    nc.sync.dma_start(
                        out=out[b * P:(b + 1) * P, fc * FC:(fc + 1) * FC], in_=so
                    )
    return nc


_N_CORES = 8
_BATCH = 8
_T = 4096
_D = 4096
_F = 4096


def _ensure_axon_hooks_module():
    """bass_utils imports antenv.axon_hooks when BASS_TRACE is set; the
    module is absent in this image.  Provide a stub so tracing degrades
    gracefully instead of crashing (a real hook may already be installed)."""
    import sys
    import types

    try:
        import antenv.axon_hooks  # noqa: F401
    except ImportError:
        mod = types.ModuleType("antenv.axon_hooks")
        mod._hook = None
        mod.set_axon_ntff_profile_hook = lambda h: setattr(mod, "_hook", h)
        mod.get_axon_ntff_profile_hook = lambda: mod._hook
        sys.modules["antenv.axon_hooks"] = mod


def kernel(x: np.ndarray, kernel: np.ndarray) -> np.ndarray:
    from concourse.bass_utils import run_bass_kernel_spmd

    _ensure_axon_hooks_module()
    install_drain_patch()
    nc = bass.Bass()
    build_bitlinear(nc, T=_T, D=_D, F=_F, TG=1024, FC=512, world=_N_CORES)
    split_multi_waits(nc)

    x = np.ascontiguousarray(np.asarray(x, dtype=np.float32))
    w = np.ascontiguousarray(np.asarray(kernel, dtype=np.float32))
    assert x.shape == (_BATCH, _T, _D) and w.shape == (_D, _F)

    wr = _D // _N_CORES
    in_maps = [
        {
            "x": x[b],
            "w": w,
            "wslice": np.ascontiguousarray(w[b * wr:(b + 1) * wr, :]),
        }
        for b in range(_N_CORES)
    ]
    res = run_bass_kernel_spmd(nc, in_maps, list(range(_N_CORES)))
    global _last_results
    _last_results = res
    return np.stack([res.results[i]["out"] for i in range(_N_CORES)], axis=0)


_last_results = None
